# revision 28
# baseline (speedup 1.0000x reference)
"""CPM3 attention kernel for 8 trn2 NeuronCores — v5.

Sharding: batch x heads (4 cores per batch, 4 heads per core, as two
head-pairs). Halves q/kv/out DMA vs pure head sharding. Host sums the
4 per-batch partial outputs (Wo row-sharded over the 4 head groups).

Design:
- host precomputes E = mask ? exp(position_bias) : 0 (fp16), since
  softmax(s + pb - inf*mask) uses exp(s + pb)*mask = exp(s) * E.
- main loop per 128k x 1024(2 heads x 512q) tile: QK matmul (the two
  heads run concurrently in separate PE row-groups) -> Exp on Scalar
  (the only main-loop Scalar work; Scalar is the critical engine at
  ~1.03us per 1024-wide Exp) -> p = e*E (fp16 DVE 2x, paired across
  two k-tiles) -> PV into fp32 ctx PSUM with a ones-column denominator.
- E streams as two [128,2048] half-tiles per group on the gpsimd+sync
  rings (1MB per (qt,hp,kg) group), prefetched 3 groups ahead; the
  first two E groups ride the scalar HWDGE ring during the prologue.
- prologue: weights on the scalar ring in parallel with the kv chunk
  stream; weight-stationary projection loops; PSUM-bank budget is
  ctx(2x2) + sc(2x2) = 8 banks, which also bounds prologue phasing.
- epilogue staggered into the next group's units: reciprocal via SBUF
  bounce (reciprocal_approx_fast from PSUM returns garbage), fp32
  per-head partition_broadcast halves (skips an fp16 cast hop), and
  normalize-mults emitted after the late p3 mults so the scheduler
  keeps PV-critical work first.
- out-projection tail batched after the main loop: hpair partials
  accumulate in PSUM, copies alternate Scalar/Vector, DMA alternates
  sync/gpsimd rings.
"""

import sys

sys.path.insert(0, "/opt/trn_rl_repo")

import numpy as np

import concourse.bass as bass
import concourse.bacc as bacc
import concourse.tile as tile
import concourse.mybir as mybir
from concourse.bass_utils import run_bass_kernel_spmd

B, L, D, H, DH = 2, 2048, 1024, 16, 64
N_CORES = 8
CPB = 4  # cores per batch
HPC = 4  # heads per core
HP = 2  # head pairs per core
QTS = 512  # q tile size
QN = L // QTS  # 4
KP = 128  # k partition tile
KN = L // KP  # 16
KTG = 4  # k tiles per DMA group
KGN = KN // KTG  # 4
DC = D // 128  # 8 contraction chunks
HVW = 2 * (DH + 1)  # 130: hv_aug columns per k-tile (2 heads x (64+ones))

F32 = mybir.dt.float32
F32R = mybir.dt.float32r
F16 = mybir.dt.float16

_CACHE: dict = {}


def _build():
    if "nc" in _CACHE:
        return _CACHE["nc"]
    nc = bacc.Bacc("TRN2", target_bir_lowering=False, debug=False, num_devices=N_CORES)

    qT = nc.dram_tensor("qT", [DC, 128, L], F16, kind="ExternalInput").ap()
    kvT = nc.dram_tensor("kvT", [DC, 128, L], F16, kind="ExternalInput").ap()
    wq = nc.dram_tensor("wq", [HP, 128, DC, 128], F16, kind="ExternalInput").ap()
    wk = nc.dram_tensor("wk", [HP, 128, DC, 128], F16, kind="ExternalInput").ap()
    wv = nc.dram_tensor("wv", [HP, 128, DC, 128], F16, kind="ExternalInput").ap()
    wo = nc.dram_tensor("wo", [HP, 128, D], F16, kind="ExternalInput").ap()
    eb = nc.dram_tensor(
        "eb", [QN, HP, KGN, 128, KTG * 2 * QTS], F16, kind="ExternalInput"
    ).ap()
    identr = nc.dram_tensor("identr", [128, 128], F32R, kind="ExternalInput").ap()
    out = nc.dram_tensor("out", [L, D], F16, kind="ExternalOutput").ap()

    with tile.TileContext(nc) as tc:
        with (
            tc.tile_pool(name="const", bufs=1) as constp,
            tc.tile_pool(name="hq", bufs=2) as hqp,
            tc.tile_pool(name="hk", bufs=2) as hkp,
            tc.tile_pool(name="hv", bufs=2) as hvp,
            tc.tile_pool(name="stage", bufs=8) as stagep,
            tc.tile_pool(name="ep", bufs=4) as epool,
            tc.tile_pool(name="p2", bufs=4) as p2p,
            tc.tile_pool(name="p3", bufs=6) as p3p,
            tc.tile_pool(name="ctxn", bufs=4) as ctxnp,
            tc.tile_pool(name="rc", bufs=2) as rcp,
            tc.tile_pool(name="outb", bufs=4) as outp,
            tc.tile_pool(name="psum", bufs=2, space=bass.MemorySpace.PSUM) as psp,
        ):
            # ---- constants (loaded between the early kv chunks) ----
            identr_t = constp.tile([128, 128], F32R, tag="identr")
            wq_t = constp.tile([128, HP, DC, 128], F16, tag="wq")
            wk_t = constp.tile([128, HP, DC, 128], F16, tag="wk")
            wv_t = constp.tile([128, HP, DC, 128], F16, tag="wv")
            wo_t = constp.tile([128, HP, D], F16, tag="wo")

            # DMA triggers cost ~650ns on the issuing engine queue. Prologue
            # chunk loads alternate the sync/gpsimd rings; weights and the
            # first E tiles ride the scalar HWDGE ring (idle until the Exps).
            trig = [nc.sync, nc.gpsimd]
            trig_i = [0]

            def dma_split(dst, src, n):
                w = L // n
                for s in range(n):
                    eng = trig[trig_i[0] % 2]
                    trig_i[0] += 1
                    eng.dma_start(
                        dst[:, s * w : (s + 1) * w], src[:, s * w : (s + 1) * w]
                    )

            # ---- prologue: kv chunks stream once; hk+hv for both head
            # pairs via weight-stationary loops; then q stream + hq.
            # weights ride the scalar HWDGE ring (idle during the prologue)
            # in parallel with the kv stream on sync+gpsimd
            nc.scalar.dma_start(wk_t[:, 0], wk[0])
            nc.scalar.dma_start(wv_t[:, 0], wv[0])
            kc = {}
            for dc in range(DC):
                kc[dc] = stagep.tile([128, L], F16, tag="stage", name=f"kc{dc}")
                dma_split(kc[dc], kvT[dc], 8 if dc < 2 else 2)
                if dc == 1:
                    nc.scalar.dma_start(wk_t[:, 1], wk[1])
                    nc.scalar.dma_start(wv_t[:, 1], wv[1])
            nc.scalar.dma_start(wq_t[:, 0], wq[0])
            nc.scalar.dma_start(wq_t[:, 1], wq[1])
            nc.scalar.dma_start(wo_t[:, 0], wo[0])
            nc.scalar.dma_start(wo_t[:, 1], wo[1])
            nc.scalar.dma_start(identr_t[:], identr[:])

            hk_sb, hq_sb, hv_sb = {}, {}, {}
            hvT = {}
            for hp in range(HP):
                hk2 = [
                    psp.tile([128, 1024], F32, tag="ctx", name=f"hk2_{hp}_{i}")
                    for i in range(2)
                ]
                hv2 = [
                    psp.tile([128, 1024], F32, tag="sc", name=f"hv2_{hp}_{i}")
                    for i in range(2)
                ]
                for dc in range(DC):
                    st, sp = (dc == 0), (dc == DC - 1)
                    for p in range(4):
                        nc.tensor.matmul(
                            hk2[p // 2][:, (p % 2) * 512 : (p % 2 + 1) * 512],
                            wk_t[:, hp, dc, :],
                            kc[dc][:, p * 512 : (p + 1) * 512],
                            start=st,
                            stop=sp,
                        )
                    for p in range(4):
                        nc.tensor.matmul(
                            hv2[p // 2][:, (p % 2) * 512 : (p % 2 + 1) * 512],
                            wv_t[:, hp, dc, :],
                            kc[dc][:, p * 512 : (p + 1) * 512],
                            start=st,
                            stop=sp,
                        )
                hk_sb[hp] = hkp.tile([128, L], F16, tag="hk", name=f"hk_sb{hp}")
                for i in range(2):
                    nc.scalar.copy(
                        hk_sb[hp][:, i * 1024 : (i + 1) * 1024], hk2[i][:]
                    )
                hvT[hp] = stagep.tile(
                    [128, L], F32R, tag="hvt", bufs=2, name=f"hvT{hp}"
                )
                nc.vector.tensor_copy(hvT[hp][:, 0:1024], hv2[0][:])
                nc.vector.tensor_copy(hvT[hp][:, 1024:2048], hv2[1][:])

                # hv_aug: transpose hvT per k-tile; ones cols prefilled
                hv_sb[hp] = hvp.tile(
                    [128, KN * HVW + 64], F16, tag="hv", name=f"hv_sb{hp}"
                )
                nc.gpsimd.memset(hv_sb[hp][:].bitcast(mybir.dt.uint16), 0x3C00)
                for kt in range(KN):
                    tp = psp.tile([128, 128], F32R, tag="sc", name=f"tp{hp}_{kt}")
                    nc.tensor.transpose(
                        tp[:], hvT[hp][:, kt * KP : (kt + 1) * KP], identr_t[:]
                    )
                    o = kt * HVW
                    nc.vector.tensor_copy(hv_sb[hp][:, o : o + DH], tp[:, 0:DH])
                    nc.vector.tensor_copy(
                        hv_sb[hp][:, o + DH + 1 : o + 2 * DH + 1], tp[:, DH:128]
                    )

            qc = {}
            for dc in range(DC):
                qc[dc] = stagep.tile([128, L], F16, tag="stage", name=f"qc{dc}")
                dma_split(qc[dc], qT[dc], 2)
            for hp in range(HP):
                hq2 = [
                    psp.tile([128, 1024], F32, tag="ctx", name=f"hq2_{hp}_{i}")
                    for i in range(2)
                ]
                for dc in range(DC):
                    for p in range(4):
                        nc.tensor.matmul(
                            hq2[p // 2][:, (p % 2) * 512 : (p % 2 + 1) * 512],
                            wq_t[:, hp, dc, :],
                            qc[dc][:, p * 512 : (p + 1) * 512],
                            start=(dc == 0),
                            stop=(dc == DC - 1),
                        )
                hq_sb[hp] = hqp.tile([128, L], F16, tag="hq", name=f"hq_sb{hp}")
                for i in range(4):
                    # only the qt0 piece gates the first QK: keep it on the
                    # scalar fast path, push the rest to the idle Vector
                    src_ap = hq2[i // 2][:, (i % 2) * 512 : (i % 2 + 1) * 512]
                    dst_ap = hq_sb[hp][:, i * 512 : (i + 1) * 512]
                    if i == 0:
                        nc.scalar.copy(dst_ap, src_ap)
                    else:
                        nc.vector.tensor_copy(dst_ap, src_ap)

            # pre-warm the gpsimd broadcast path (first call pays a library
            # load) on a scratch tile during the prologue
            warm_src = rcp.tile([1, 1024], F32, tag="rcf", name="warm_src")
            nc.gpsimd.memset(warm_src[:], 0.0)
            warm_bc = rcp.tile([128, 1024], F32, tag="bcsb", name="warm_bc")
            nc.gpsimd.partition_broadcast(warm_bc[:], warm_src[:])

            # ---- E stream prefetch bookkeeping ----
            egroups = [
                (qt, hp, kg)
                for qt in range(QN)
                for hp in range(HP)
                for kg in range(KGN)
            ]
            e_tiles = {}

            def ensure_e(gi):
                if gi >= len(egroups) or gi in e_tiles:
                    return
                qt, hp, kg = egroups[gi]
                t = epool.tile(
                    [128, KTG, 2 * QTS], F16, tag="e", name=f"e_{qt}_{hp}_{kg}"
                )
                src = eb[qt, hp, kg]
                if gi < 2:
                    # prologue prefetch on the scalar ring, clear of kv/q
                    nc.scalar.dma_start(t[:, 0:2], src[:, 0:2048])
                    nc.scalar.dma_start(t[:, 2:4], src[:, 2048:4096])
                else:
                    nc.gpsimd.dma_start(t[:, 0:2], src[:, 0:2048])
                    nc.sync.dma_start(t[:, 2:4], src[:, 2048:4096])
                e_tiles[gi] = t

            ensure_e(0)
            ensure_e(1)

            # ---- per-group epilogue: normalization only ----
            class Epi:
                def __init__(self, qt, hp, ctx2):
                    self.qt, self.hp, self.ctx2 = qt, hp, ctx2
                    self.bc = None
                    self.ctxn = None

            ctxn_done = {}  # (qt, hp) -> ctxn tile
            pending_tail = []

            def epi_step(st, step):
                qt, hp, ctx2 = st.qt, st.hp, st.ctx2
                if step == 0:
                    st.dsb = rcp.tile([1, 1024], F32, tag="dsb", name=f"dsb{hp}_{qt}")
                    nc.vector.tensor_copy(st.dsb[:], ctx2[DH : DH + 1, :])
                elif step == 1:
                    st.rcf = rcp.tile([1, 1024], F32, tag="rcf", name=f"rcf{hp}_{qt}")
                    nc.vector.reciprocal_approx_fast(st.rcf[:], st.dsb[:])
                elif step == 2 or step == 3:
                    h = step - 2
                    if h == 0:
                        st.bc = rcp.tile(
                            [128, 1024], F32, tag="bcsb", name=f"bc{hp}_{qt}"
                        )
                    nc.gpsimd.partition_broadcast(
                        st.bc[:, h * QTS : (h + 1) * QTS],
                        st.rcf[:, h * QTS : (h + 1) * QTS],
                    )
                else:
                    h = step - 4
                    if h == 0:
                        st.ctxn = ctxnp.tile(
                            [128, QTS], F16, tag="ctxn", bufs=8, name=f"ctxn{hp}_{qt}"
                        )
                    nc.vector.tensor_tensor(
                        st.ctxn[h * DH : (h + 1) * DH, :],
                        ctx2[0:DH, h * QTS : (h + 1) * QTS],
                        st.bc[h * DH : (h + 1) * DH, h * QTS : (h + 1) * QTS],
                        mybir.AluOpType.mult,
                    )
                    if h == 1:
                        ctxn_done[qt, hp] = st.ctxn
                        if hp == 1:
                            for qs in range(4):
                                pending_tail.append((qt, qs))

            EPI_AT = {4: [0], 5: [1], 6: [2], 7: [3], 13: [4], 15: [5]}

            # ---- out-projection tail round (batched after the main loop) ----
            tail_i = [0]

            def emit_tail(qt, qs):
                i = tail_i[0]
                tail_i[0] += 1
                op2 = psp.tile(
                    [128, 1024], F32, tag="sc" if i % 2 == 0 else "ctx",
                    name=f"op_{qt}_{qs}"
                )
                for hp in range(HP):
                    for oh in range(2):
                        nc.tensor.matmul(
                            op2[:, oh * 512 : (oh + 1) * 512],
                            ctxn_done[qt, hp][:, qs * 128 : (qs + 1) * 128],
                            wo_t[:, hp, oh * 512 : (oh + 1) * 512],
                            start=(hp == 0),
                            stop=(hp == 1),
                        )
                ob = outp.tile([128, D], F16, tag="outb", bufs=6, name=f"ob_{qt}_{qs}")
                if i % 2 == 0:
                    nc.scalar.copy(ob[:], op2[:])
                else:
                    nc.vector.tensor_copy(ob[:], op2[:])
                r0 = qt * QTS + qs * 128
                eng = nc.sync if i % 2 == 0 else nc.gpsimd
                eng.dma_start(out[r0 : r0 + 128, :], ob[:])

            # ---- main loop ----
            def emit_pv(hp, kt, p3, pe, ctx2):
                for h in range(2):
                    o = kt * HVW + h * (DH + 1)
                    nc.tensor.matmul(
                        ctx2[:, h * QTS : (h + 1) * QTS],
                        hv_sb[hp][:, o : o + 128],
                        p3[:, pe, h * QTS : (h + 1) * QTS],
                        start=(kt == 0),
                        stop=(kt == KN - 1),
                    )

            pending_pv = []
            cur_epi = None
            for qt in range(QN):
                for hp in range(HP):
                    ctx2 = psp.tile(
                        [128, 1024], F32, tag="ctx", name=f"ctx2_{qt}_{hp}"
                    )
                    for kg in range(KGN):
                        gi = (qt * HP + hp) * KGN + kg
                        ensure_e(gi + 1)
                        ensure_e(gi + 2)
                        ensure_e(gi + 3)
                        e4 = e_tiles[gi]
                        p2 = None
                        for ki in range(KTG):
                            kt = kg * KTG + ki
                            unit = kg * KTG + ki
                            sc2 = psp.tile(
                                [128, 1024], F32, tag="sc", name=f"sc{qt}_{hp}_{kt}"
                            )
                            # bias the scheduler to order QKs ahead of
                            # same-window PVs in the PE queue: the Exp
                            # stream gates on QK, never on PV
                            with tc.high_priority(offset=8):
                                for h in range(2):
                                    nc.tensor.matmul(
                                        sc2[:, h * QTS : (h + 1) * QTS],
                                        hk_sb[hp][
                                            h * DH : (h + 1) * DH,
                                            kt * KP : (kt + 1) * KP,
                                        ],
                                        hq_sb[hp][
                                            h * DH : (h + 1) * DH,
                                            qt * QTS : (qt + 1) * QTS,
                                        ],
                                        start=True,
                                        stop=True,
                                    )
                            while len(pending_pv) >= 4:
                                emit_pv(*pending_pv.pop(0))
                            if ki % 2 == 0:
                                p2 = p2p.tile(
                                    [128, 2, 1024], F16, tag="p2", name=f"p2_{qt}_{hp}_{kt}"
                                )
                            nc.scalar.activation(
                                p2[:, ki % 2], sc2[:], mybir.ActivationFunctionType.Exp
                            )
                            if ki % 2 == 1:
                                p3 = p3p.tile(
                                    [128, 2, 1024], F16, tag="p3", name=f"p3_{qt}_{hp}_{kt}"
                                )
                                nc.vector.tensor_tensor(
                                    p3[:], p2[:], e4[:, ki - 1 : ki + 1],
                                    mybir.AluOpType.mult,
                                )
                                pending_pv.append((hp, kt - 1, p3, 0, ctx2))
                                pending_pv.append((hp, kt, p3, 1, ctx2))
                            if cur_epi is not None and unit in EPI_AT:
                                for s in EPI_AT[unit]:
                                    epi_step(cur_epi, s)
                    cur_epi = Epi(qt, hp, ctx2)
            for item in pending_pv:
                emit_pv(*item)
            for s in range(6):
                epi_step(cur_epi, s)
            while pending_tail:
                emit_tail(*pending_tail.pop(0))

    nc.compile()
    _CACHE["nc"] = nc
    return nc


def _prep_core(core, position_bias, Wq, Wk, Wv, Wo, shared):
    bc = core // CPB
    h0 = (core % CPB) * HPC
    rows = slice(h0 * DH, (h0 + HPC) * DH)  # 256 rows

    def packw(w, scale=1.0):
        wr = w[rows].T * scale  # [D, 256]
        return np.ascontiguousarray(
            np.stack(
                [
                    wr[:, hp * 128 : (hp + 1) * 128]
                    .reshape(DC, 128, 128)
                    .transpose(1, 0, 2)
                    for hp in range(HP)
                ]
            )
        ).astype(np.float16)

    # E = mask ? exp(position_bias) : 0, packed [qt, hp, kg, kp, (ki h2 qf)]
    expb = np.exp(position_bias[h0 : h0 + HPC], dtype=np.float32)  # [4, q, k]
    ec = (expb * shared["maskf"][bc][None]).astype(np.float16)
    ec = ec.reshape(HP, 2, QN, QTS, KGN, KTG, KP)
    ep = np.ascontiguousarray(ec.transpose(2, 0, 4, 6, 5, 1, 3)).reshape(
        QN, HP, KGN, KP, KTG * 2 * QTS
    )
    wor = Wo[:, rows].T  # [256, D]
    return {
        "qT": shared["qT"][bc],
        "kvT": shared["kvT"][bc],
        "identr": shared["identr"],
        "wq": packw(Wq, 1.0 / np.sqrt(DH)),
        "wk": packw(Wk),
        "wv": packw(Wv),
        "wo": np.ascontiguousarray(
            np.stack([wor[hp * 128 : (hp + 1) * 128] for hp in range(HP)])
        ).astype(np.float16),
        "eb": ep,
    }


def _prep_shared(query, key_value, mask):
    qTp = np.ascontiguousarray(
        query.reshape(B, L, DC, 128).transpose(0, 2, 3, 1)
    ).astype(np.float16)
    kvTp = np.ascontiguousarray(
        key_value.reshape(B, L, DC, 128).transpose(0, 2, 3, 1)
    ).astype(np.float16)
    return {
        "qT": qTp,
        "kvT": kvTp,
        "maskf": np.asarray(mask, dtype=bool).astype(np.float32),
        "identr": np.eye(128, dtype=np.float32),
    }


def kernel(query, key_value, mask, position_bias, Wq, Wk, Wv, Wo, _trace=False):
    query = np.asarray(query, dtype=np.float32)
    key_value = np.asarray(key_value, dtype=np.float32)
    mask = np.asarray(mask)
    position_bias = np.asarray(position_bias, dtype=np.float32)
    Wq = np.asarray(Wq, dtype=np.float32)
    Wk = np.asarray(Wk, dtype=np.float32)
    Wv = np.asarray(Wv, dtype=np.float32)
    Wo = np.asarray(Wo, dtype=np.float32)

    nc = _build()
    shared = _prep_shared(query, key_value, mask)
    in_maps = [
        _prep_core(c, position_bias, Wq, Wk, Wv, Wo, shared) for c in range(N_CORES)
    ]
    res = run_bass_kernel_spmd(nc, in_maps, list(range(N_CORES)), trace=_trace)
    _CACHE["last_result"] = res
    full = np.zeros((B, L, D), np.float64)
    for c in range(N_CORES):
        full[c // CPB] += res.results[c]["out"].astype(np.float64)
    return full.astype(np.float32)


# revision 29
# speedup vs baseline: 1.0158x; 1.0158x over previous
"""CPM3 attention kernel for 8 trn2 NeuronCores — v5.

Sharding: batch x heads (4 cores per batch, 4 heads per core, as two
head-pairs). Halves q/kv/out DMA vs pure head sharding. Host sums the
4 per-batch partial outputs (Wo row-sharded over the 4 head groups).

Design:
- host precomputes E = mask ? exp(position_bias) : 0 (fp16), since
  softmax(s + pb - inf*mask) uses exp(s + pb)*mask = exp(s) * E.
- main loop per 128k x 1024(2 heads x 512q) tile: QK matmul (the two
  heads run concurrently in separate PE row-groups) -> Exp on Scalar
  (the only main-loop Scalar work; Scalar is the critical engine at
  ~1.03us per 1024-wide Exp) -> p = e*E (fp16 DVE 2x, paired across
  two k-tiles) -> PV into fp32 ctx PSUM with a ones-column denominator.
- E streams as two [128,2048] half-tiles per group on the gpsimd+sync
  rings (1MB per (qt,hp,kg) group), prefetched 3 groups ahead; the
  first two E groups ride the scalar HWDGE ring during the prologue.
- prologue: weights on the scalar ring in parallel with the kv chunk
  stream; weight-stationary projection loops; PSUM-bank budget is
  ctx(2x2) + sc(2x2) = 8 banks, which also bounds prologue phasing.
- epilogue staggered into the next group's units: reciprocal via SBUF
  bounce (reciprocal_approx_fast from PSUM returns garbage), fp32
  per-head partition_broadcast halves (skips an fp16 cast hop), and
  normalize-mults emitted after the late p3 mults so the scheduler
  keeps PV-critical work first.
- out-projection tail batched after the main loop: hpair partials
  accumulate in PSUM, copies alternate Scalar/Vector, DMA alternates
  sync/gpsimd rings.
"""

import sys

sys.path.insert(0, "/opt/trn_rl_repo")

import numpy as np

import concourse.bass as bass
import concourse.bacc as bacc
import concourse.tile as tile
import concourse.mybir as mybir
from concourse.bass_utils import run_bass_kernel_spmd

B, L, D, H, DH = 2, 2048, 1024, 16, 64
N_CORES = 8
CPB = 4  # cores per batch
HPC = 4  # heads per core
HP = 2  # head pairs per core
QTS = 512  # q tile size
QN = L // QTS  # 4
KP = 128  # k partition tile
KN = L // KP  # 16
KTG = 4  # k tiles per DMA group
KGN = KN // KTG  # 4
DC = D // 128  # 8 contraction chunks
HVW = 2 * (DH + 1)  # 130: hv_aug columns per k-tile (2 heads x (64+ones))

F32 = mybir.dt.float32
F32R = mybir.dt.float32r
F16 = mybir.dt.float16

_CACHE: dict = {}


def _build():
    if "nc" in _CACHE:
        return _CACHE["nc"]
    nc = bacc.Bacc("TRN2", target_bir_lowering=False, debug=False, num_devices=N_CORES)

    qT = nc.dram_tensor("qT", [DC, 128, L], F16, kind="ExternalInput").ap()
    kvT = nc.dram_tensor("kvT", [DC, 128, L], F16, kind="ExternalInput").ap()
    wq = nc.dram_tensor("wq", [HP, 128, DC, 128], F16, kind="ExternalInput").ap()
    wk = nc.dram_tensor("wk", [HP, 128, DC, 128], F16, kind="ExternalInput").ap()
    wv = nc.dram_tensor("wv", [HP, 128, DC, 128], F16, kind="ExternalInput").ap()
    wo = nc.dram_tensor("wo", [HP, 128, D], F16, kind="ExternalInput").ap()
    eb = nc.dram_tensor(
        "eb", [QN, HP, KGN, 128, KTG * 2 * QTS], F16, kind="ExternalInput"
    ).ap()
    identr = nc.dram_tensor("identr", [128, 128], F32R, kind="ExternalInput").ap()
    out = nc.dram_tensor("out", [L, D], F16, kind="ExternalOutput").ap()

    with tile.TileContext(nc) as tc:
        with (
            tc.tile_pool(name="const", bufs=1) as constp,
            tc.tile_pool(name="hq", bufs=2) as hqp,
            tc.tile_pool(name="hk", bufs=2) as hkp,
            tc.tile_pool(name="hv", bufs=2) as hvp,
            tc.tile_pool(name="stage", bufs=8) as stagep,
            tc.tile_pool(name="ep", bufs=4) as epool,
            tc.tile_pool(name="p2", bufs=4) as p2p,
            tc.tile_pool(name="p3", bufs=6) as p3p,
            tc.tile_pool(name="ctxn", bufs=4) as ctxnp,
            tc.tile_pool(name="rc", bufs=2) as rcp,
            tc.tile_pool(name="outb", bufs=4) as outp,
            tc.tile_pool(name="psum", bufs=2, space=bass.MemorySpace.PSUM) as psp,
        ):
            # ---- constants (loaded between the early kv chunks) ----
            identr_t = constp.tile([128, 128], F32R, tag="identr")
            wq_t = constp.tile([128, HP, DC, 128], F16, tag="wq")
            wk_t = constp.tile([128, HP, DC, 128], F16, tag="wk")
            wv_t = constp.tile([128, HP, DC, 128], F16, tag="wv")
            wo_t = constp.tile([128, HP, D], F16, tag="wo")

            # DMA triggers cost ~650ns on the issuing engine queue. Prologue
            # chunk loads alternate the sync/gpsimd rings; weights and the
            # first E tiles ride the scalar HWDGE ring (idle until the Exps).
            trig = [nc.sync, nc.gpsimd]
            trig_i = [0]

            def dma_split(dst, src, n):
                w = L // n
                for s in range(n):
                    eng = trig[trig_i[0] % 2]
                    trig_i[0] += 1
                    eng.dma_start(
                        dst[:, s * w : (s + 1) * w], src[:, s * w : (s + 1) * w]
                    )

            # ---- prologue: kv chunks stream once; hk+hv for both head
            # pairs via weight-stationary loops; then q stream + hq.
            # weights ride the scalar HWDGE ring (idle during the prologue)
            # in parallel with the kv stream on sync+gpsimd
            nc.scalar.dma_start(wk_t[:, 0], wk[0])
            nc.scalar.dma_start(wv_t[:, 0], wv[0])
            kc = {}
            for dc in range(DC):
                kc[dc] = stagep.tile([128, L], F16, tag="stage", name=f"kc{dc}")
                dma_split(kc[dc], kvT[dc], 8 if dc < 2 else 2)
                if dc == 1:
                    nc.scalar.dma_start(wk_t[:, 1], wk[1])
                    nc.scalar.dma_start(wv_t[:, 1], wv[1])
            nc.scalar.dma_start(wq_t[:, 0], wq[0])
            nc.scalar.dma_start(wq_t[:, 1], wq[1])
            nc.scalar.dma_start(wo_t[:, 0], wo[0])
            nc.scalar.dma_start(wo_t[:, 1], wo[1])
            nc.scalar.dma_start(identr_t[:], identr[:])

            hk_sb, hq_sb, hv_sb = {}, {}, {}
            hvT = {}
            for hp in range(HP):
                hk2 = [
                    psp.tile([128, 1024], F32, tag="ctx", name=f"hk2_{hp}_{i}")
                    for i in range(2)
                ]
                hv2 = [
                    psp.tile([128, 1024], F32, tag="sc", name=f"hv2_{hp}_{i}")
                    for i in range(2)
                ]
                for dc in range(DC):
                    st, sp = (dc == 0), (dc == DC - 1)
                    for p in range(4):
                        nc.tensor.matmul(
                            hk2[p // 2][:, (p % 2) * 512 : (p % 2 + 1) * 512],
                            wk_t[:, hp, dc, :],
                            kc[dc][:, p * 512 : (p + 1) * 512],
                            start=st,
                            stop=sp,
                        )
                    for p in range(4):
                        nc.tensor.matmul(
                            hv2[p // 2][:, (p % 2) * 512 : (p % 2 + 1) * 512],
                            wv_t[:, hp, dc, :],
                            kc[dc][:, p * 512 : (p + 1) * 512],
                            start=st,
                            stop=sp,
                        )
                hk_sb[hp] = hkp.tile([128, L], F16, tag="hk", name=f"hk_sb{hp}")
                for i in range(2):
                    nc.scalar.copy(
                        hk_sb[hp][:, i * 1024 : (i + 1) * 1024], hk2[i][:]
                    )
                hvT[hp] = stagep.tile(
                    [128, L], F32R, tag="hvt", bufs=2, name=f"hvT{hp}"
                )
                nc.vector.tensor_copy(hvT[hp][:, 0:1024], hv2[0][:])
                nc.vector.tensor_copy(hvT[hp][:, 1024:2048], hv2[1][:])

                # hv_aug: transpose hvT per k-tile; ones cols prefilled
                hv_sb[hp] = hvp.tile(
                    [128, KN * HVW + 64], F16, tag="hv", name=f"hv_sb{hp}"
                )
                nc.gpsimd.memset(hv_sb[hp][:].bitcast(mybir.dt.uint16), 0x3C00)
                for kt in range(KN):
                    tp = psp.tile([128, 128], F32R, tag="sc", name=f"tp{hp}_{kt}")
                    nc.tensor.transpose(
                        tp[:], hvT[hp][:, kt * KP : (kt + 1) * KP], identr_t[:]
                    )
                    o = kt * HVW
                    nc.vector.tensor_copy(hv_sb[hp][:, o : o + DH], tp[:, 0:DH])
                    nc.vector.tensor_copy(
                        hv_sb[hp][:, o + DH + 1 : o + 2 * DH + 1], tp[:, DH:128]
                    )

            qc = {}
            for dc in range(DC):
                qc[dc] = stagep.tile([128, L], F16, tag="stage", name=f"qc{dc}")
                dma_split(qc[dc], qT[dc], 2)
            for hp in range(HP):
                hq2 = [
                    psp.tile([128, 1024], F32, tag="ctx", name=f"hq2_{hp}_{i}")
                    for i in range(2)
                ]
                for dc in range(DC):
                    for p in range(4):
                        nc.tensor.matmul(
                            hq2[p // 2][:, (p % 2) * 512 : (p % 2 + 1) * 512],
                            wq_t[:, hp, dc, :],
                            qc[dc][:, p * 512 : (p + 1) * 512],
                            start=(dc == 0),
                            stop=(dc == DC - 1),
                        )
                hq_sb[hp] = hqp.tile([128, L], F16, tag="hq", name=f"hq_sb{hp}")
                for i in range(4):
                    nc.scalar.copy(
                        hq_sb[hp][:, i * 512 : (i + 1) * 512],
                        hq2[i // 2][:, (i % 2) * 512 : (i % 2 + 1) * 512],
                    )

            # pre-warm the gpsimd broadcast path (first call pays a library
            # load) on a scratch tile during the prologue
            warm_src = rcp.tile([1, 1024], F32, tag="rcf", name="warm_src")
            nc.gpsimd.memset(warm_src[:], 0.0)
            warm_bc = rcp.tile([128, 1024], F32, tag="bcsb", name="warm_bc")
            nc.gpsimd.partition_broadcast(warm_bc[:], warm_src[:])

            # ---- E stream prefetch bookkeeping ----
            egroups = [
                (qt, hp, kg)
                for qt in range(QN)
                for hp in range(HP)
                for kg in range(KGN)
            ]
            e_tiles = {}

            def ensure_e(gi):
                if gi >= len(egroups) or gi in e_tiles:
                    return
                qt, hp, kg = egroups[gi]
                t = epool.tile(
                    [128, KTG, 2 * QTS], F16, tag="e", name=f"e_{qt}_{hp}_{kg}"
                )
                src = eb[qt, hp, kg]
                if gi < 2:
                    # prologue prefetch on the scalar ring, clear of kv/q
                    nc.scalar.dma_start(t[:, 0:2], src[:, 0:2048])
                    nc.scalar.dma_start(t[:, 2:4], src[:, 2048:4096])
                else:
                    nc.gpsimd.dma_start(t[:, 0:2], src[:, 0:2048])
                    nc.sync.dma_start(t[:, 2:4], src[:, 2048:4096])
                e_tiles[gi] = t

            ensure_e(0)
            ensure_e(1)

            # ---- per-group epilogue: normalization only ----
            class Epi:
                def __init__(self, qt, hp, ctx2):
                    self.qt, self.hp, self.ctx2 = qt, hp, ctx2
                    self.bc = None
                    self.ctxn = None

            ctxn_done = {}  # (qt, hp) -> ctxn tile
            pending_tail = []

            def epi_step(st, step):
                qt, hp, ctx2 = st.qt, st.hp, st.ctx2
                if step == 0:
                    st.dsb = rcp.tile([1, 1024], F32, tag="dsb", name=f"dsb{hp}_{qt}")
                    nc.vector.tensor_copy(st.dsb[:], ctx2[DH : DH + 1, :])
                elif step == 1:
                    st.rcf = rcp.tile([1, 1024], F32, tag="rcf", name=f"rcf{hp}_{qt}")
                    nc.vector.reciprocal_approx_fast(st.rcf[:], st.dsb[:])
                elif step == 2 or step == 3:
                    h = step - 2
                    if h == 0:
                        st.bc = rcp.tile(
                            [128, 1024], F32, tag="bcsb", name=f"bc{hp}_{qt}"
                        )
                    nc.gpsimd.partition_broadcast(
                        st.bc[:, h * QTS : (h + 1) * QTS],
                        st.rcf[:, h * QTS : (h + 1) * QTS],
                    )
                else:
                    h = step - 4
                    if h == 0:
                        st.ctxn = ctxnp.tile(
                            [128, QTS], F16, tag="ctxn", bufs=8, name=f"ctxn{hp}_{qt}"
                        )
                    nc.vector.tensor_tensor(
                        st.ctxn[h * DH : (h + 1) * DH, :],
                        ctx2[0:DH, h * QTS : (h + 1) * QTS],
                        st.bc[h * DH : (h + 1) * DH, h * QTS : (h + 1) * QTS],
                        mybir.AluOpType.mult,
                    )
                    if h == 1:
                        ctxn_done[qt, hp] = st.ctxn
                        if hp == 1:
                            for qs in range(4):
                                pending_tail.append((qt, qs))

            EPI_AT = {4: [0], 5: [1], 6: [2], 7: [3], 13: [4], 15: [5]}

            # ---- out-projection tail round (batched after the main loop) ----
            tail_i = [0]

            def emit_tail(qt, qs):
                i = tail_i[0]
                tail_i[0] += 1
                op2 = psp.tile(
                    [128, 1024], F32, tag="sc" if i % 2 == 0 else "ctx",
                    name=f"op_{qt}_{qs}"
                )
                for hp in range(HP):
                    for oh in range(2):
                        nc.tensor.matmul(
                            op2[:, oh * 512 : (oh + 1) * 512],
                            ctxn_done[qt, hp][:, qs * 128 : (qs + 1) * 128],
                            wo_t[:, hp, oh * 512 : (oh + 1) * 512],
                            start=(hp == 0),
                            stop=(hp == 1),
                        )
                ob = outp.tile([128, D], F16, tag="outb", bufs=6, name=f"ob_{qt}_{qs}")
                if i % 2 == 0:
                    nc.scalar.copy(ob[:], op2[:])
                else:
                    nc.vector.tensor_copy(ob[:], op2[:])
                r0 = qt * QTS + qs * 128
                eng = nc.sync if i % 2 == 0 else nc.gpsimd
                eng.dma_start(out[r0 : r0 + 128, :], ob[:])

            # ---- main loop ----
            def emit_pv(hp, kt, p3, pe, ctx2):
                for h in range(2):
                    o = kt * HVW + h * (DH + 1)
                    nc.tensor.matmul(
                        ctx2[:, h * QTS : (h + 1) * QTS],
                        hv_sb[hp][:, o : o + 128],
                        p3[:, pe, h * QTS : (h + 1) * QTS],
                        start=(kt == 0),
                        stop=(kt == KN - 1),
                    )

            pending_pv = []
            cur_epi = None
            for qt in range(QN):
                for hp in range(HP):
                    ctx2 = psp.tile(
                        [128, 1024], F32, tag="ctx", name=f"ctx2_{qt}_{hp}"
                    )
                    for kg in range(KGN):
                        gi = (qt * HP + hp) * KGN + kg
                        ensure_e(gi + 1)
                        ensure_e(gi + 2)
                        ensure_e(gi + 3)
                        e4 = e_tiles[gi]
                        p2 = None
                        for ki in range(KTG):
                            kt = kg * KTG + ki
                            unit = kg * KTG + ki
                            sc2 = psp.tile(
                                [128, 1024], F32, tag="sc", name=f"sc{qt}_{hp}_{kt}"
                            )
                            # bias the scheduler to order QKs ahead of
                            # same-window PVs in the PE queue: the Exp
                            # stream gates on QK, never on PV
                            with tc.high_priority(offset=8):
                                for h in range(2):
                                    nc.tensor.matmul(
                                        sc2[:, h * QTS : (h + 1) * QTS],
                                        hk_sb[hp][
                                            h * DH : (h + 1) * DH,
                                            kt * KP : (kt + 1) * KP,
                                        ],
                                        hq_sb[hp][
                                            h * DH : (h + 1) * DH,
                                            qt * QTS : (qt + 1) * QTS,
                                        ],
                                        start=True,
                                        stop=True,
                                    )
                            while len(pending_pv) >= 4:
                                emit_pv(*pending_pv.pop(0))
                            if ki % 2 == 0:
                                p2 = p2p.tile(
                                    [128, 2, 1024], F16, tag="p2", name=f"p2_{qt}_{hp}_{kt}"
                                )
                            nc.scalar.activation(
                                p2[:, ki % 2], sc2[:], mybir.ActivationFunctionType.Exp
                            )
                            if ki % 2 == 1:
                                p3 = p3p.tile(
                                    [128, 2, 1024], F16, tag="p3", name=f"p3_{qt}_{hp}_{kt}"
                                )
                                nc.vector.tensor_tensor(
                                    p3[:], p2[:], e4[:, ki - 1 : ki + 1],
                                    mybir.AluOpType.mult,
                                )
                                pending_pv.append((hp, kt - 1, p3, 0, ctx2))
                                pending_pv.append((hp, kt, p3, 1, ctx2))
                            if cur_epi is not None and unit in EPI_AT:
                                for s in EPI_AT[unit]:
                                    epi_step(cur_epi, s)
                    cur_epi = Epi(qt, hp, ctx2)
            for item in pending_pv:
                emit_pv(*item)
            for s in range(6):
                epi_step(cur_epi, s)
            while pending_tail:
                emit_tail(*pending_tail.pop(0))

    nc.compile()
    _CACHE["nc"] = nc
    return nc


def _prep_core(core, position_bias, Wq, Wk, Wv, Wo, shared):
    bc = core // CPB
    h0 = (core % CPB) * HPC
    rows = slice(h0 * DH, (h0 + HPC) * DH)  # 256 rows

    def packw(w, scale=1.0):
        wr = w[rows].T * scale  # [D, 256]
        return np.ascontiguousarray(
            np.stack(
                [
                    wr[:, hp * 128 : (hp + 1) * 128]
                    .reshape(DC, 128, 128)
                    .transpose(1, 0, 2)
                    for hp in range(HP)
                ]
            )
        ).astype(np.float16)

    # E = mask ? exp(position_bias) : 0, packed [qt, hp, kg, kp, (ki h2 qf)]
    expb = np.exp(position_bias[h0 : h0 + HPC], dtype=np.float32)  # [4, q, k]
    ec = (expb * shared["maskf"][bc][None]).astype(np.float16)
    ec = ec.reshape(HP, 2, QN, QTS, KGN, KTG, KP)
    ep = np.ascontiguousarray(ec.transpose(2, 0, 4, 6, 5, 1, 3)).reshape(
        QN, HP, KGN, KP, KTG * 2 * QTS
    )
    wor = Wo[:, rows].T  # [256, D]
    return {
        "qT": shared["qT"][bc],
        "kvT": shared["kvT"][bc],
        "identr": shared["identr"],
        "wq": packw(Wq, 1.0 / np.sqrt(DH)),
        "wk": packw(Wk),
        "wv": packw(Wv),
        "wo": np.ascontiguousarray(
            np.stack([wor[hp * 128 : (hp + 1) * 128] for hp in range(HP)])
        ).astype(np.float16),
        "eb": ep,
    }


def _prep_shared(query, key_value, mask):
    qTp = np.ascontiguousarray(
        query.reshape(B, L, DC, 128).transpose(0, 2, 3, 1)
    ).astype(np.float16)
    kvTp = np.ascontiguousarray(
        key_value.reshape(B, L, DC, 128).transpose(0, 2, 3, 1)
    ).astype(np.float16)
    return {
        "qT": qTp,
        "kvT": kvTp,
        "maskf": np.asarray(mask, dtype=bool).astype(np.float32),
        "identr": np.eye(128, dtype=np.float32),
    }


def kernel(query, key_value, mask, position_bias, Wq, Wk, Wv, Wo, _trace=False):
    query = np.asarray(query, dtype=np.float32)
    key_value = np.asarray(key_value, dtype=np.float32)
    mask = np.asarray(mask)
    position_bias = np.asarray(position_bias, dtype=np.float32)
    Wq = np.asarray(Wq, dtype=np.float32)
    Wk = np.asarray(Wk, dtype=np.float32)
    Wv = np.asarray(Wv, dtype=np.float32)
    Wo = np.asarray(Wo, dtype=np.float32)

    nc = _build()
    shared = _prep_shared(query, key_value, mask)
    in_maps = [
        _prep_core(c, position_bias, Wq, Wk, Wv, Wo, shared) for c in range(N_CORES)
    ]
    res = run_bass_kernel_spmd(nc, in_maps, list(range(N_CORES)), trace=_trace)
    _CACHE["last_result"] = res
    full = np.zeros((B, L, D), np.float64)
    for c in range(N_CORES):
        full[c // CPB] += res.results[c]["out"].astype(np.float64)
    return full.astype(np.float32)


# revision 30
# speedup vs baseline: 1.0247x; 1.0087x over previous
"""CPM3 attention kernel for 8 trn2 NeuronCores — v5.

Sharding: batch x heads (4 cores per batch, 4 heads per core, as two
head-pairs). Halves q/kv/out DMA vs pure head sharding. Host sums the
4 per-batch partial outputs (Wo row-sharded over the 4 head groups).

Design:
- host precomputes E = mask ? exp(position_bias) : 0 (fp16), since
  softmax(s + pb - inf*mask) uses exp(s + pb)*mask = exp(s) * E.
- main loop per 128k x 1024(2 heads x 512q) tile: QK matmul (the two
  heads run concurrently in separate PE row-groups; emitted under
  tc.high_priority so the scheduler orders QKs ahead of PVs — the Exp
  stream gates on QK, never on PV) -> Exp on Scalar (the only
  main-loop Scalar work; Scalar is the critical engine at ~1.03us per
  1024-wide Exp) -> p = e*E (fp16 DVE 2x, paired across two k-tiles)
  -> PV into fp32 ctx PSUM with a ones-column denominator.
- E streams as two [128,2048] half-tiles per group on the gpsimd+sync
  rings (1MB per (qt,hp,kg) group), prefetched 3 groups ahead; the
  first two E groups ride the scalar HWDGE ring during the prologue.
- prologue: weights on the scalar ring in parallel with the kv chunk
  stream; weight-stationary projection loops; PSUM-bank budget is
  ctx(2x2) + sc(2x2) = 8 banks, which also bounds prologue phasing.
- epilogue staggered into the next group's units: reciprocal via SBUF
  bounce (reciprocal_approx_fast from PSUM returns garbage), fp32
  per-head partition_broadcast halves (skips an fp16 cast hop), and
  normalize-mults emitted after the late p3 mults so the scheduler
  keeps PV-critical work first.
- out-projection tail batched after the main loop: hpair partials
  accumulate in PSUM, copies alternate Scalar/Vector, DMA alternates
  sync/gpsimd rings.
"""

import sys

sys.path.insert(0, "/opt/trn_rl_repo")

import numpy as np

import concourse.bass as bass
import concourse.bacc as bacc
import concourse.tile as tile
import concourse.mybir as mybir
from concourse.bass_utils import run_bass_kernel_spmd

B, L, D, H, DH = 2, 2048, 1024, 16, 64
N_CORES = 8
CPB = 4  # cores per batch
HPC = 4  # heads per core
HP = 2  # head pairs per core
QTS = 512  # q tile size
QN = L // QTS  # 4
KP = 128  # k partition tile
KN = L // KP  # 16
KTG = 4  # k tiles per DMA group
KGN = KN // KTG  # 4
DC = D // 128  # 8 contraction chunks
HVW = 2 * (DH + 1)  # 130: hv_aug columns per k-tile (2 heads x (64+ones))

F32 = mybir.dt.float32
F32R = mybir.dt.float32r
F16 = mybir.dt.float16

_CACHE: dict = {}


def _build():
    if "nc" in _CACHE:
        return _CACHE["nc"]
    nc = bacc.Bacc("TRN2", target_bir_lowering=False, debug=False, num_devices=N_CORES)

    qT = nc.dram_tensor("qT", [DC, 128, L], F16, kind="ExternalInput").ap()
    kvT = nc.dram_tensor("kvT", [DC, 128, L], F16, kind="ExternalInput").ap()
    wq = nc.dram_tensor("wq", [HP, 128, DC, 128], F16, kind="ExternalInput").ap()
    wk = nc.dram_tensor("wk", [HP, 128, DC, 128], F16, kind="ExternalInput").ap()
    wv = nc.dram_tensor("wv", [HP, 128, DC, 128], F16, kind="ExternalInput").ap()
    wo = nc.dram_tensor("wo", [HP, 128, D], F16, kind="ExternalInput").ap()
    eb = nc.dram_tensor(
        "eb", [QN, HP, KGN, 128, KTG * 2 * QTS], F16, kind="ExternalInput"
    ).ap()
    identr = nc.dram_tensor("identr", [128, 128], F32R, kind="ExternalInput").ap()
    out = nc.dram_tensor("out", [L, D], F16, kind="ExternalOutput").ap()

    with tile.TileContext(nc) as tc:
        with (
            tc.tile_pool(name="const", bufs=1) as constp,
            tc.tile_pool(name="hq", bufs=2) as hqp,
            tc.tile_pool(name="hk", bufs=2) as hkp,
            tc.tile_pool(name="hv", bufs=2) as hvp,
            tc.tile_pool(name="stage", bufs=8) as stagep,
            tc.tile_pool(name="ep", bufs=4) as epool,
            tc.tile_pool(name="p2", bufs=4) as p2p,
            tc.tile_pool(name="p3", bufs=6) as p3p,
            tc.tile_pool(name="ctxn", bufs=4) as ctxnp,
            tc.tile_pool(name="rc", bufs=2) as rcp,
            tc.tile_pool(name="outb", bufs=4) as outp,
            tc.tile_pool(name="psum", bufs=2, space=bass.MemorySpace.PSUM) as psp,
        ):
            # ---- constants (loaded between the early kv chunks) ----
            identr_t = constp.tile([128, 128], F32R, tag="identr")
            wq_t = constp.tile([128, HP, DC, 128], F16, tag="wq")
            wk_t = constp.tile([128, HP, DC, 128], F16, tag="wk")
            wv_t = constp.tile([128, HP, DC, 128], F16, tag="wv")
            wo_t = constp.tile([128, HP, D], F16, tag="wo")

            # DMA triggers cost ~650ns on the issuing engine queue. Prologue
            # chunk loads alternate the sync/gpsimd rings; weights and the
            # first E tiles ride the scalar HWDGE ring (idle until the Exps).
            trig = [nc.sync, nc.gpsimd]
            trig_i = [0]

            def dma_split(dst, src, n):
                w = L // n
                for s in range(n):
                    eng = trig[trig_i[0] % 2]
                    trig_i[0] += 1
                    eng.dma_start(
                        dst[:, s * w : (s + 1) * w], src[:, s * w : (s + 1) * w]
                    )

            # ---- prologue: kv chunks stream once; hk+hv for both head
            # pairs via weight-stationary loops; then q stream + hq.
            # weights ride the scalar HWDGE ring (idle during the prologue)
            # in parallel with the kv stream on sync+gpsimd
            nc.scalar.dma_start(wk_t[:, 0], wk[0])
            nc.scalar.dma_start(wv_t[:, 0], wv[0])
            kc = {}
            for dc in range(DC):
                kc[dc] = stagep.tile([128, L], F16, tag="stage", name=f"kc{dc}")
                dma_split(kc[dc], kvT[dc], 8 if dc < 2 else 2)
                if dc == 1:
                    nc.scalar.dma_start(wk_t[:, 1], wk[1])
                    nc.scalar.dma_start(wv_t[:, 1], wv[1])
            nc.scalar.dma_start(wq_t[:, 0], wq[0])
            nc.scalar.dma_start(wq_t[:, 1], wq[1])
            nc.scalar.dma_start(wo_t[:, 0], wo[0])
            nc.scalar.dma_start(wo_t[:, 1], wo[1])
            nc.scalar.dma_start(identr_t[:], identr[:])

            hk_sb, hq_sb, hv_sb = {}, {}, {}
            hvT = {}
            for hp in range(HP):
                hk2 = [
                    psp.tile([128, 1024], F32, tag="ctx", name=f"hk2_{hp}_{i}")
                    for i in range(2)
                ]
                hv2 = [
                    psp.tile([128, 1024], F32, tag="sc", name=f"hv2_{hp}_{i}")
                    for i in range(2)
                ]
                for dc in range(DC):
                    st, sp = (dc == 0), (dc == DC - 1)
                    for p in range(4):
                        nc.tensor.matmul(
                            hk2[p // 2][:, (p % 2) * 512 : (p % 2 + 1) * 512],
                            wk_t[:, hp, dc, :],
                            kc[dc][:, p * 512 : (p + 1) * 512],
                            start=st,
                            stop=sp,
                        )
                    for p in range(4):
                        nc.tensor.matmul(
                            hv2[p // 2][:, (p % 2) * 512 : (p % 2 + 1) * 512],
                            wv_t[:, hp, dc, :],
                            kc[dc][:, p * 512 : (p + 1) * 512],
                            start=st,
                            stop=sp,
                        )
                hk_sb[hp] = hkp.tile([128, L], F16, tag="hk", name=f"hk_sb{hp}")
                for i in range(2):
                    nc.scalar.copy(
                        hk_sb[hp][:, i * 1024 : (i + 1) * 1024], hk2[i][:]
                    )
                hvT[hp] = stagep.tile(
                    [128, L], F32R, tag="hvt", bufs=2, name=f"hvT{hp}"
                )
                nc.vector.tensor_copy(hvT[hp][:, 0:1024], hv2[0][:])
                nc.vector.tensor_copy(hvT[hp][:, 1024:2048], hv2[1][:])

                # hv_aug: transpose hvT per k-tile; ones cols prefilled
                hv_sb[hp] = hvp.tile(
                    [128, KN * HVW + 64], F16, tag="hv", name=f"hv_sb{hp}"
                )
                nc.gpsimd.memset(hv_sb[hp][:].bitcast(mybir.dt.uint16), 0x3C00)
                for kt in range(KN):
                    tp = psp.tile([128, 128], F32R, tag="sc", name=f"tp{hp}_{kt}")
                    nc.tensor.transpose(
                        tp[:], hvT[hp][:, kt * KP : (kt + 1) * KP], identr_t[:]
                    )
                    o = kt * HVW
                    nc.vector.tensor_copy(hv_sb[hp][:, o : o + DH], tp[:, 0:DH])
                    nc.vector.tensor_copy(
                        hv_sb[hp][:, o + DH + 1 : o + 2 * DH + 1], tp[:, DH:128]
                    )

            qc = {}
            for dc in range(DC):
                qc[dc] = stagep.tile([128, L], F16, tag="stage", name=f"qc{dc}")
                dma_split(qc[dc], qT[dc], 2)
            for hp in range(HP):
                hq2 = [
                    psp.tile([128, 1024], F32, tag="ctx", name=f"hq2_{hp}_{i}")
                    for i in range(2)
                ]
                for dc in range(DC):
                    for p in range(4):
                        nc.tensor.matmul(
                            hq2[p // 2][:, (p % 2) * 512 : (p % 2 + 1) * 512],
                            wq_t[:, hp, dc, :],
                            qc[dc][:, p * 512 : (p + 1) * 512],
                            start=(dc == 0),
                            stop=(dc == DC - 1),
                        )
                hq_sb[hp] = hqp.tile([128, L], F16, tag="hq", name=f"hq_sb{hp}")
                for i in range(4):
                    nc.scalar.copy(
                        hq_sb[hp][:, i * 512 : (i + 1) * 512],
                        hq2[i // 2][:, (i % 2) * 512 : (i % 2 + 1) * 512],
                    )

            # pre-warm the gpsimd broadcast path (first call pays a library
            # load) on a scratch tile during the prologue
            warm_src = rcp.tile([1, 1024], F32, tag="rcf", name="warm_src")
            nc.gpsimd.memset(warm_src[:], 0.0)
            warm_bc = rcp.tile([128, 1024], F32, tag="bcsb", name="warm_bc")
            nc.gpsimd.partition_broadcast(warm_bc[:], warm_src[:])

            # ---- E stream prefetch bookkeeping ----
            egroups = [
                (qt, hp, kg)
                for qt in range(QN)
                for hp in range(HP)
                for kg in range(KGN)
            ]
            e_tiles = {}

            def ensure_e(gi):
                if gi >= len(egroups) or gi in e_tiles:
                    return
                qt, hp, kg = egroups[gi]
                t = epool.tile(
                    [128, KTG, 2 * QTS], F16, tag="e", name=f"e_{qt}_{hp}_{kg}"
                )
                src = eb[qt, hp, kg]
                if gi < 2:
                    # prologue prefetch on the scalar ring, clear of kv/q
                    nc.scalar.dma_start(t[:, 0:2], src[:, 0:2048])
                    nc.scalar.dma_start(t[:, 2:4], src[:, 2048:4096])
                else:
                    nc.gpsimd.dma_start(t[:, 0:2], src[:, 0:2048])
                    nc.sync.dma_start(t[:, 2:4], src[:, 2048:4096])
                e_tiles[gi] = t

            ensure_e(0)
            ensure_e(1)

            # ---- per-group epilogue: normalization only ----
            class Epi:
                def __init__(self, qt, hp, ctx2):
                    self.qt, self.hp, self.ctx2 = qt, hp, ctx2
                    self.bc = None
                    self.ctxn = None

            ctxn_done = {}  # (qt, hp) -> ctxn tile
            pending_tail = []

            def epi_step(st, step):
                qt, hp, ctx2 = st.qt, st.hp, st.ctx2
                if step == 0:
                    st.dsb = rcp.tile([1, 1024], F32, tag="dsb", name=f"dsb{hp}_{qt}")
                    nc.vector.tensor_copy(st.dsb[:], ctx2[DH : DH + 1, :])
                elif step == 1:
                    st.rcf = rcp.tile([1, 1024], F32, tag="rcf", name=f"rcf{hp}_{qt}")
                    nc.vector.reciprocal_approx_fast(st.rcf[:], st.dsb[:])
                elif step == 2 or step == 3:
                    h = step - 2
                    if h == 0:
                        st.bc = rcp.tile(
                            [128, 1024], F32, tag="bcsb", name=f"bc{hp}_{qt}"
                        )
                    nc.gpsimd.partition_broadcast(
                        st.bc[:, h * QTS : (h + 1) * QTS],
                        st.rcf[:, h * QTS : (h + 1) * QTS],
                    )
                else:
                    h = step - 4
                    if h == 0:
                        st.ctxn = ctxnp.tile(
                            [128, QTS], F16, tag="ctxn", bufs=8, name=f"ctxn{hp}_{qt}"
                        )
                    nc.vector.tensor_tensor(
                        st.ctxn[h * DH : (h + 1) * DH, :],
                        ctx2[0:DH, h * QTS : (h + 1) * QTS],
                        st.bc[h * DH : (h + 1) * DH, h * QTS : (h + 1) * QTS],
                        mybir.AluOpType.mult,
                    )
                    if h == 1:
                        ctxn_done[qt, hp] = st.ctxn
                        if hp == 1:
                            for qs in range(4):
                                pending_tail.append((qt, qs))

            EPI_AT = {4: [0], 5: [1], 6: [2], 7: [3], 13: [4], 15: [5]}

            # ---- out-projection tail round (batched after the main loop) ----
            tail_i = [0]

            def emit_tail(qt, qs):
                i = tail_i[0]
                tail_i[0] += 1
                op2 = psp.tile(
                    [128, 1024], F32, tag="sc" if i % 2 == 0 else "ctx",
                    name=f"op_{qt}_{qs}"
                )
                for hp in range(HP):
                    for oh in range(2):
                        nc.tensor.matmul(
                            op2[:, oh * 512 : (oh + 1) * 512],
                            ctxn_done[qt, hp][:, qs * 128 : (qs + 1) * 128],
                            wo_t[:, hp, oh * 512 : (oh + 1) * 512],
                            start=(hp == 0),
                            stop=(hp == 1),
                        )
                ob = outp.tile([128, D], F16, tag="outb", bufs=6, name=f"ob_{qt}_{qs}")
                if i % 2 == 0:
                    nc.scalar.copy(ob[:], op2[:])
                else:
                    nc.vector.tensor_copy(ob[:], op2[:])
                r0 = qt * QTS + qs * 128
                eng = nc.sync if i % 2 == 0 else nc.gpsimd
                eng.dma_start(out[r0 : r0 + 128, :], ob[:])

            # ---- main loop ----
            def emit_pv(hp, kt, p3, pe, ctx2):
                for h in range(2):
                    o = kt * HVW + h * (DH + 1)
                    nc.tensor.matmul(
                        ctx2[:, h * QTS : (h + 1) * QTS],
                        hv_sb[hp][:, o : o + 128],
                        p3[:, pe, h * QTS : (h + 1) * QTS],
                        start=(kt == 0),
                        stop=(kt == KN - 1),
                    )

            pending_pv = []
            cur_epi = None
            for qt in range(QN):
                for hp in range(HP):
                    ctx2 = psp.tile(
                        [128, 1024], F32, tag="ctx", name=f"ctx2_{qt}_{hp}"
                    )
                    for kg in range(KGN):
                        gi = (qt * HP + hp) * KGN + kg
                        ensure_e(gi + 1)
                        ensure_e(gi + 2)
                        ensure_e(gi + 3)
                        e4 = e_tiles[gi]
                        p2 = None
                        for ki in range(KTG):
                            kt = kg * KTG + ki
                            unit = kg * KTG + ki
                            sc2 = psp.tile(
                                [128, 1024], F32, tag="sc", name=f"sc{qt}_{hp}_{kt}"
                            )
                            # bias the scheduler to order QKs ahead of
                            # same-window PVs in the PE queue: the Exp
                            # stream gates on QK, never on PV
                            with tc.high_priority(offset=8):
                                for h in range(2):
                                    nc.tensor.matmul(
                                        sc2[:, h * QTS : (h + 1) * QTS],
                                        hk_sb[hp][
                                            h * DH : (h + 1) * DH,
                                            kt * KP : (kt + 1) * KP,
                                        ],
                                        hq_sb[hp][
                                            h * DH : (h + 1) * DH,
                                            qt * QTS : (qt + 1) * QTS,
                                        ],
                                        start=True,
                                        stop=True,
                                    )
                            while len(pending_pv) >= 4:
                                emit_pv(*pending_pv.pop(0))
                            if ki % 2 == 0:
                                p2 = p2p.tile(
                                    [128, 2, 1024], F16, tag="p2", name=f"p2_{qt}_{hp}_{kt}"
                                )
                            nc.scalar.activation(
                                p2[:, ki % 2], sc2[:], mybir.ActivationFunctionType.Exp
                            )
                            if ki % 2 == 1:
                                p3 = p3p.tile(
                                    [128, 2, 1024], F16, tag="p3", name=f"p3_{qt}_{hp}_{kt}"
                                )
                                nc.vector.tensor_tensor(
                                    p3[:], p2[:], e4[:, ki - 1 : ki + 1],
                                    mybir.AluOpType.mult,
                                )
                                pending_pv.append((hp, kt - 1, p3, 0, ctx2))
                                pending_pv.append((hp, kt, p3, 1, ctx2))
                            if cur_epi is not None and unit in EPI_AT:
                                for s in EPI_AT[unit]:
                                    epi_step(cur_epi, s)
                    cur_epi = Epi(qt, hp, ctx2)
            for item in pending_pv:
                emit_pv(*item)
            for s in range(6):
                epi_step(cur_epi, s)
            while pending_tail:
                emit_tail(*pending_tail.pop(0))

    nc.compile()
    _CACHE["nc"] = nc
    return nc


def _prep_core(core, position_bias, Wq, Wk, Wv, Wo, shared):
    bc = core // CPB
    h0 = (core % CPB) * HPC
    rows = slice(h0 * DH, (h0 + HPC) * DH)  # 256 rows

    def packw(w, scale=1.0):
        wr = w[rows].T * scale  # [D, 256]
        return np.ascontiguousarray(
            np.stack(
                [
                    wr[:, hp * 128 : (hp + 1) * 128]
                    .reshape(DC, 128, 128)
                    .transpose(1, 0, 2)
                    for hp in range(HP)
                ]
            )
        ).astype(np.float16)

    # E = mask ? exp(position_bias) : 0, packed [qt, hp, kg, kp, (ki h2 qf)]
    expb = np.exp(position_bias[h0 : h0 + HPC], dtype=np.float32)  # [4, q, k]
    ec = (expb * shared["maskf"][bc][None]).astype(np.float16)
    ec = ec.reshape(HP, 2, QN, QTS, KGN, KTG, KP)
    ep = np.ascontiguousarray(ec.transpose(2, 0, 4, 6, 5, 1, 3)).reshape(
        QN, HP, KGN, KP, KTG * 2 * QTS
    )
    wor = Wo[:, rows].T  # [256, D]
    return {
        "qT": shared["qT"][bc],
        "kvT": shared["kvT"][bc],
        "identr": shared["identr"],
        "wq": packw(Wq, 1.0 / np.sqrt(DH)),
        "wk": packw(Wk),
        "wv": packw(Wv),
        "wo": np.ascontiguousarray(
            np.stack([wor[hp * 128 : (hp + 1) * 128] for hp in range(HP)])
        ).astype(np.float16),
        "eb": ep,
    }


def _prep_shared(query, key_value, mask):
    qTp = np.ascontiguousarray(
        query.reshape(B, L, DC, 128).transpose(0, 2, 3, 1)
    ).astype(np.float16)
    kvTp = np.ascontiguousarray(
        key_value.reshape(B, L, DC, 128).transpose(0, 2, 3, 1)
    ).astype(np.float16)
    return {
        "qT": qTp,
        "kvT": kvTp,
        "maskf": np.asarray(mask, dtype=bool).astype(np.float32),
        "identr": np.eye(128, dtype=np.float32),
    }


def kernel(query, key_value, mask, position_bias, Wq, Wk, Wv, Wo, _trace=False):
    query = np.asarray(query, dtype=np.float32)
    key_value = np.asarray(key_value, dtype=np.float32)
    mask = np.asarray(mask)
    position_bias = np.asarray(position_bias, dtype=np.float32)
    Wq = np.asarray(Wq, dtype=np.float32)
    Wk = np.asarray(Wk, dtype=np.float32)
    Wv = np.asarray(Wv, dtype=np.float32)
    Wo = np.asarray(Wo, dtype=np.float32)

    nc = _build()
    shared = _prep_shared(query, key_value, mask)
    in_maps = [
        _prep_core(c, position_bias, Wq, Wk, Wv, Wo, shared) for c in range(N_CORES)
    ]
    res = run_bass_kernel_spmd(nc, in_maps, list(range(N_CORES)), trace=_trace)
    _CACHE["last_result"] = res
    full = np.zeros((B, L, D), np.float64)
    for c in range(N_CORES):
        full[c // CPB] += res.results[c]["out"].astype(np.float64)
    return full.astype(np.float32)
